# revision 25
# baseline (speedup 1.0000x reference)
"""ConvAttn Trainium2 kernel (nn_ConvAttn_61503931678811).

Reference, per (b, h):
    X = Q.reshape(b, h*d, l)  (raw reshape -> channel i = h*64 + l//32,
                               scrambled "length" j = (l%32)*64 + d)
    q = conv1d(X, Wq) (same-pad, k=3); k = conv1d(K-reshape, Wk)
    scores = q @ k^T / 8 ; attn = softmax(scores)
    context = einsum('bhqk,bhvd->bhqd', attn, V)
            = rowsum(attn)[q] * V.sum(axis=v)[d]  (k and v sum independently)

Sharding: core c -> batch b = c//4, head group hg = c%4 (4 heads/core).
Conv out-channels sliced per head group; conv input replicated per batch.

Per-core pipeline (all phases share one PSUM pool, 8 banks):
  conv (x2):  gpsimd cast-DMA X/W^T to f32r; Y[oc,j] = sum W^T X (f32r
              matmuls, PSUM acc over 24 (ic,t) terms); ACT evacuates PSUM;
              PE transposes + DVE cast-copies build per-head-pair
              A_T[d, hh, lhi, llo] (= q^T[d, l], l = lhi*32 + llo) in f32r.
  attention:  per (head, q-chunk of 128): 4 f32r matmuls -> scores PSUM
              [128, 2048]; ACT exp(0.125*s) with fused row-sum; DVE recip +
              scale; DMA out 1 MiB.
  context:    per head: Vsum via ones-matmul, broadcast via ones-matmul,
              ctx = bcast * denom * recip (the einsum quirk makes context
              rank-1 in q), DMA out.
"""

import numpy as np

import concourse.bacc as bacc
import concourse.mybir as mybir
from concourse import tile
from concourse.bass_utils import run_bass_kernel_spmd

F32 = mybir.dt.float32
F32R = mybir.dt.float32r
EXP = mybir.ActivationFunctionType.Exp
MULT = mybir.AluOpType.mult

B, H, L, DK = 2, 16, 2048, 64
C = H * DK
NHC = 4
OC = NHC * DK
KER = 3
ICCH = C // 128
LCH = L // 512
QCH = L // 128
KCH = L // 512

_NC_CACHE = {}

import os as _os
TR_DELAY = int(_os.environ.get("K_TR_DELAY", "0"))
W2 = int(_os.environ.get("K_W2", "0"))
PSB = int(_os.environ.get("K_PSB", "4"))


def _build_nc():
    nc = bacc.Bacc(None, target_bir_lowering=False)

    xq = nc.dram_tensor("xq", [C, L], F32, kind="ExternalInput")
    xk = nc.dram_tensor("xk", [C, L], F32, kind="ExternalInput")
    wq = nc.dram_tensor("wq", [C, KER, OC], F32, kind="ExternalInput")
    wk = nc.dram_tensor("wk", [C, KER, OC], F32, kind="ExternalInput")
    v = nc.dram_tensor("v", [NHC, L, DK], F32, kind="ExternalInput")
    attn = nc.dram_tensor("attn", [NHC, L, L], F32, kind="ExternalOutput")
    ctx = nc.dram_tensor("ctx", [NHC, L, DK], F32, kind="ExternalOutput")

    ident = nc.inline_tensor(np.eye(128, dtype=np.float32), name="ident128")

    with tile.TileContext(nc) as tc:
        with (
            tc.tile_pool(name="at", bufs=1) as at_pool,
            tc.tile_pool(name="small", bufs=1) as small,
            tc.tile_pool(name="ps", bufs=1, space="PSUM") as ps_pool,
            tc.tile_pool(name="pb", bufs=1) as pb_pool,
        ):
            # A_T per head pair: [64 d, 2 hh, 64 lhi, 32 llo] f32r
            at_q = [at_pool.tile([64, 2, 64, 32], F32R, tag=f"atq{hp}", name=f"atq{hp}") for hp in range(2)]
            at_k = [at_pool.tile([64, 2, 64, 32], F32R, tag=f"atk{hp}", name=f"atk{hp}") for hp in range(2)]

            id_sb = small.tile([128, 128], F32, tag="ident", name="id_sb")
            nc.sync.dma_start(id_sb[:], ident[:])
            ones_c = small.tile([128, 1], F32, tag="ones_c", name="ones_c")
            nc.vector.memset(ones_c[:], 1.0)
            ones_r = small.tile([1, 128], F32, tag="ones_r", name="ones_r")
            nc.vector.memset(ones_r[:], 1.0)
            scratch1 = small.tile([128, 1], F32, tag="scr1", name="scratch1")
            # warm up the exp table load while conv runs
            nc.scalar.activation(scratch1[:], ones_c[:], EXP, bias=0.0, scale=1.0)
            # per-(head, q-chunk) denominator/reciprocal tiles: separate tiles
            # so successive chunks don't serialize on a shared tile
            _dent = [
                [small.tile([128, 8], F32, tag=f"den{h}_{g}", name=f"den{h}_{g}") for g in range(QCH // 8)]
                for h in range(NHC)
            ]
            _rect = [
                [small.tile([128, 8], F32, tag=f"rec{h}_{g}", name=f"rec{h}_{g}") for g in range(QCH // 8)]
                for h in range(NHC)
            ]
            rs_den = [[_dent[h][qc // 8][:, qc % 8 : qc % 8 + 1] for qc in range(QCH)] for h in range(NHC)]
            rs_rec = [[_rect[h][qc // 8][:, qc % 8 : qc % 8 + 1] for qc in range(QCH)] for h in range(NHC)]

            # ---------------- convs ----------------
            def emit_transposes(y_sb, at_dst, hp, lp, j):
                for lo in range(j * 8, (j + 1) * 8):
                    llo = lp * 16 + lo
                    pt = ps_pool.tile([64, 2, 64], F32, tag="trps", name="trps", bufs=int(_os.environ.get("K_TPS", "2")))
                    nc.tensor.transpose(
                        pt[:].rearrange("p a b -> p (a b)"),
                        y_sb[:, lo * 64 : (lo + 1) * 64],
                        id_sb[:],
                    )
                    nc.vector.tensor_copy(at_dst[hp][:, :, :, llo], pt[:])

            def conv_pass(x_t, w_t, at_dst, hp):
                # lc-pair x i-outer nesting: each x tile's reads for this pass
                # finish within its i-slice, so x slots free progressively.
                for lp in range(LCH // 2):
                    y_sb = y_pool.tile([128, L // 2], F32, tag="y", name="y_sb", bufs=3)
                    ps = [
                        ps_pool.tile([128, 512], F32, tag="convps", name="convps", bufs=int(_os.environ.get("K_CPS", "2")))
                        for _ in range(2)
                    ]
                    for i in range(ICCH):
                        for t in range(KER):
                            for j in range(2):
                                lc = lp * 2 + j
                                nc.tensor.matmul(
                                    ps[j][:],
                                    w_t[i][:, t, hp * 128 : (hp + 1) * 128],
                                    x_t[i][:, lc * 512 + t : lc * 512 + t + 512],
                                    start=(i == 0 and t == 0),
                                    stop=(i == ICCH - 1 and t == KER - 1),
                                )
                    for j in range(2):
                        nc.scalar.copy(y_sb[:, j * 512 : (j + 1) * 512], ps[j][:])
                        emit_transposes(y_sb, at_dst, hp, lp, j)

            def load_xw(x_dram, w_dram, x_t, w_t):
                for i in range(ICCH):
                    t = xw_pool.tile([128, L + 2], F32R, tag=f"x{i}", name=f"xt{i}")
                    nc.vector.memset(t[:, 0:1].bitcast(F32), 0.0)
                    nc.vector.memset(t[:, L + 1 : L + 2].bitcast(F32), 0.0)
                    nc.gpsimd.dma_start(t[:, 1 : L + 1], x_dram[i * 128 : (i + 1) * 128, :])
                    x_t.append(t)
                    tw = xw_pool.tile([128, KER, OC], F32R, tag=f"w{i}", name=f"wt{i}", bufs=2 if i < W2 else 1)
                    nc.gpsimd.dma_start(tw[:], w_dram[i * 128 : (i + 1) * 128])
                    w_t.append(tw)

            def attn_head(h):
                hp, hh = h // 2, h % 2
                qh = at_q[hp][:, hh].rearrange("p a b -> p (a b)")
                kh = at_k[hp][:, hh].rearrange("p a b -> p (a b)")
                for qc in range(QCH):
                    p_sb = pb_pool.tile([128, L], F32, tag="p", name="p_sb", bufs=PSB)
                    halves = []
                    for half in range(2):
                        ps_s = ps_pool.tile([128, L // 2], F32, tag="sc", name="sc", bufs=2)
                        for kc in range(2):
                            nc.tensor.matmul(
                                ps_s[:, kc * 512 : (kc + 1) * 512],
                                qh[:, qc * 128 : (qc + 1) * 128],
                                kh[:, (half * 2 + kc) * 512 : (half * 2 + kc + 1) * 512],
                                start=True,
                                stop=True,
                            )
                        scr = small.tile([128, 1], F32, tag=f"scr_{half}", name=f"escr{half}", bufs=4)
                        nc.scalar.activation(
                            p_sb[:, half * (L // 2) : (half + 1) * (L // 2)],
                            ps_s[:], EXP,
                            bias=0.0, scale=0.125,
                            accum_out=scr[:],
                        )
                        halves.append(scr)
                    den = rs_den[h][qc]
                    rec = rs_rec[h][qc]
                    nc.vector.tensor_add(den, halves[0][:], halves[1][:])
                    nc.vector.reciprocal(rec, den)
                    nc.vector.tensor_scalar_mul(p_sb[:], p_sb[:], rec)
                    nc.sync.dma_start(attn[h, qc * 128 : (qc + 1) * 128, :], p_sb[:])

            with (
                tc.tile_pool(name="xw", bufs=1) as xw_pool,
                tc.tile_pool(name="yb", bufs=1) as y_pool,
            ):
                xq_t, wq_t = [], []
                load_xw(xq, wq, xq_t, wq_t)
                conv_pass(xq_t, wq_t, at_q, 0)
                conv_pass(xq_t, wq_t, at_q, 1)
                xk_t, wk_t = [], []
                load_xw(xk, wk, xk_t, wk_t)
                conv_pass(xk_t, wk_t, at_k, 0)
                # attention for pair 0 starts while conv-K pair 1 finishes
                attn_head(0)
                conv_pass(xk_t, wk_t, at_k, 1)
                attn_head(1)
            with tc.tile_pool(name="pc", bufs=1) as pc_pool:
                v_sb = [pc_pool.tile([128, 16, DK], F32, tag=f"v{h}", name=f"vsb{h}") for h in range(NHC)]
                for h in range(NHC):
                    nc.gpsimd.dma_start(v_sb[h][:], v[h].rearrange("(n p) d -> p n d", p=128))

                def ctx_head(h):
                    vs_ps = ps_pool.tile([1, DK], F32, tag="convps", name="vs_ps", bufs=int(_os.environ.get("K_CPS", "2")))
                    for n in range(16):
                        nc.tensor.matmul(
                            vs_ps[:], ones_c[:], v_sb[h][:, n, :],
                            start=(n == 0), stop=(n == 15),
                        )
                    vs_sb = pc_pool.tile([1, DK], F32, tag="vs_sb", name="vs_sb", bufs=2)
                    nc.scalar.copy(vs_sb[:], vs_ps[:])
                    bc_ps = ps_pool.tile([128, DK], F32, tag="trps", name="bc_ps", bufs=int(_os.environ.get("K_TPS", "2")))
                    nc.tensor.matmul(bc_ps[:], ones_r[:], vs_sb[:], start=True, stop=True)
                    bc_sb = pc_pool.tile([128, DK], F32, tag="bc_sb", name="bc_sb", bufs=2)
                    nc.vector.tensor_copy(bc_sb[:], bc_ps[:])
                    ctx_sb = pc_pool.tile([128, QCH, DK], F32, tag="ctx_sb", name="ctx_sb", bufs=2)
                    for qc in range(QCH):
                        nc.vector.tensor_scalar(
                            ctx_sb[:, qc, :], bc_sb[:],
                            rs_den[h][qc], rs_rec[h][qc],
                            op0=MULT, op1=MULT,
                        )
                    nc.sync.dma_start(ctx[h].rearrange("(n p) d -> p n d", p=128), ctx_sb[:])

                attn_head(2)
                ctx_head(0)
                ctx_head(1)
                attn_head(3)
                ctx_head(2)
                ctx_head(3)

    nc.compile()
    return nc


def kernel(Q, K, V, Wq, Wk):
    Q = np.ascontiguousarray(np.asarray(Q), dtype=np.float32)
    K = np.ascontiguousarray(np.asarray(K), dtype=np.float32)
    V = np.ascontiguousarray(np.asarray(V), dtype=np.float32)
    Wq = np.asarray(Wq)
    Wk = np.asarray(Wk)

    if "nc" not in _NC_CACHE:
        _NC_CACHE["nc"] = _build_nc()
    nc = _NC_CACHE["nc"]

    Qr = Q.reshape(B, C, L)
    Kr = K.reshape(B, C, L)
    wq_t = [
        np.ascontiguousarray(np.transpose(Wq[hg * OC : (hg + 1) * OC], (1, 2, 0)), dtype=np.float32)
        for hg in range(4)
    ]
    wk_t = [
        np.ascontiguousarray(np.transpose(Wk[hg * OC : (hg + 1) * OC], (1, 2, 0)), dtype=np.float32)
        for hg in range(4)
    ]

    in_maps = []
    for c in range(8):
        b, hg = c // 4, c % 4
        in_maps.append(
            {
                "xq": Qr[b],
                "xk": Kr[b],
                "wq": wq_t[hg],
                "wk": wk_t[hg],
                "v": np.ascontiguousarray(V[b, hg * NHC : (hg + 1) * NHC]),
            }
        )

    results = run_bass_kernel_spmd(nc, in_maps, core_ids=list(range(8))).results

    attn_full = np.empty((B, H, L, L), dtype=np.float32)
    ctx_full = np.empty((B, H, L, DK), dtype=np.float32)
    for c, r in enumerate(results):
        b, hg = c // 4, c % 4
        attn_full[b, hg * NHC : (hg + 1) * NHC] = r["attn"]
        ctx_full[b, hg * NHC : (hg + 1) * NHC] = r["ctx"]
    return (ctx_full, attn_full)



# revision 26
# speedup vs baseline: 1.0060x; 1.0060x over previous
"""ConvAttn Trainium2 kernel (nn_ConvAttn_61503931678811).

Reference, per (b, h):
    X = Q.reshape(b, h*d, l)  (raw reshape -> channel i = h*64 + l//32,
                               scrambled "length" j = (l%32)*64 + d)
    q = conv1d(X, Wq) (same-pad, k=3); k = conv1d(K-reshape, Wk)
    scores = q @ k^T / 8 ; attn = softmax(scores)
    context = einsum('bhqk,bhvd->bhqd', attn, V)
            = rowsum(attn)[q] * V.sum(axis=v)[d]  (k and v sum independently)

Sharding: core c -> batch b = c//4, head group hg = c%4 (4 heads/core).
Conv out-channels sliced per head group; conv input replicated per batch.

Per-core pipeline (all phases share one PSUM pool, 8 banks):
  conv (x2):  gpsimd cast-DMA X/W^T to f32r; Y[oc,j] = sum W^T X (f32r
              matmuls, PSUM acc over 24 (ic,t) terms); ACT evacuates PSUM;
              PE transposes + DVE cast-copies build per-head-pair
              A_T[d, hh, lhi, llo] (= q^T[d, l], l = lhi*32 + llo) in f32r.
  attention:  per (head, q-chunk of 128): 4 f32r matmuls -> scores PSUM
              [128, 2048]; ACT exp(0.125*s) with fused row-sum; DVE recip +
              scale; DMA out 1 MiB.
  context:    per head: Vsum via ones-matmul, broadcast via ones-matmul,
              ctx = bcast * denom * recip (the einsum quirk makes context
              rank-1 in q), DMA out.
"""

import numpy as np

import concourse.bacc as bacc
import concourse.mybir as mybir
from concourse import tile
from concourse.bass_utils import run_bass_kernel_spmd

F32 = mybir.dt.float32
F32R = mybir.dt.float32r
EXP = mybir.ActivationFunctionType.Exp
MULT = mybir.AluOpType.mult

B, H, L, DK = 2, 16, 2048, 64
C = H * DK
NHC = 4
OC = NHC * DK
KER = 3
ICCH = C // 128
LCH = L // 512
QCH = L // 128
KCH = L // 512

_NC_CACHE = {}

import os as _os
TR_DELAY = int(_os.environ.get("K_TR_DELAY", "0"))
W2 = int(_os.environ.get("K_W2", "0"))
PSB = int(_os.environ.get("K_PSB", "4"))


def _build_nc():
    nc = bacc.Bacc(None, target_bir_lowering=False)

    xq = nc.dram_tensor("xq", [C, L], F32, kind="ExternalInput")
    xk = nc.dram_tensor("xk", [C, L], F32, kind="ExternalInput")
    wq = nc.dram_tensor("wq", [C, KER, OC], F32, kind="ExternalInput")
    wk = nc.dram_tensor("wk", [C, KER, OC], F32, kind="ExternalInput")
    v = nc.dram_tensor("v", [NHC, L, DK], F32, kind="ExternalInput")
    attn = nc.dram_tensor("attn", [NHC, L, L], F32, kind="ExternalOutput")
    ctx = nc.dram_tensor("ctx", [NHC, L, DK], F32, kind="ExternalOutput")

    ident = nc.inline_tensor(np.eye(128, dtype=np.float32), name="ident128")

    with tile.TileContext(nc) as tc:
        with (
            tc.tile_pool(name="at", bufs=1) as at_pool,
            tc.tile_pool(name="small", bufs=1) as small,
            tc.tile_pool(name="ps", bufs=1, space="PSUM") as ps_pool,
            tc.tile_pool(name="pb", bufs=1) as pb_pool,
        ):
            # A_T per head pair: [64 d, 2 hh, 64 lhi, 32 llo] f32r
            at_q = [at_pool.tile([64, 2, 64, 32], F32R, tag=f"atq{hp}", name=f"atq{hp}") for hp in range(2)]
            at_k = [at_pool.tile([64, 2, 64, 32], F32R, tag=f"atk{hp}", name=f"atk{hp}") for hp in range(2)]

            id_sb = small.tile([128, 128], F32, tag="ident", name="id_sb")
            nc.sync.dma_start(id_sb[:], ident[:])
            ones_c = small.tile([128, 1], F32, tag="ones_c", name="ones_c")
            nc.vector.memset(ones_c[:], 1.0)
            ones_r = small.tile([1, 128], F32, tag="ones_r", name="ones_r")
            nc.vector.memset(ones_r[:], 1.0)
            scratch1 = small.tile([128, 1], F32, tag="scr1", name="scratch1")
            # warm up the exp table load while conv runs
            nc.scalar.activation(scratch1[:], ones_c[:], EXP, bias=0.0, scale=1.0)
            # per-(head, q-chunk) denominator/reciprocal tiles: separate tiles
            # so successive chunks don't serialize on a shared tile
            _dent = [
                [small.tile([128, 8], F32, tag=f"den{h}_{g}", name=f"den{h}_{g}") for g in range(QCH // 8)]
                for h in range(NHC)
            ]
            _rect = [
                [small.tile([128, 8], F32, tag=f"rec{h}_{g}", name=f"rec{h}_{g}") for g in range(QCH // 8)]
                for h in range(NHC)
            ]
            rs_den = [[_dent[h][qc // 8][:, qc % 8 : qc % 8 + 1] for qc in range(QCH)] for h in range(NHC)]
            rs_rec = [[_rect[h][qc // 8][:, qc % 8 : qc % 8 + 1] for qc in range(QCH)] for h in range(NHC)]

            # ---------------- convs ----------------
            def emit_transposes(y_sb, at_dst, hp, lp, j):
                for lo in range(j * 8, (j + 1) * 8):
                    llo = lp * 16 + lo
                    pt = ps_pool.tile([64, 2, 64], F32, tag="trps", name="trps", bufs=int(_os.environ.get("K_TPS", "2")))
                    nc.tensor.transpose(
                        pt[:].rearrange("p a b -> p (a b)"),
                        y_sb[:, lo * 64 : (lo + 1) * 64],
                        id_sb[:],
                    )
                    nc.vector.tensor_copy(at_dst[hp][:, :, :, llo], pt[:])

            def conv_pass(x_t, w_t, at_dst, hp):
                # lc-pair x i-outer nesting: each x tile's reads for this pass
                # finish within its i-slice, so x slots free progressively.
                for lp in range(LCH // 2):
                    y_sb = y_pool.tile([128, L // 2], F32, tag="y", name="y_sb", bufs=3)
                    ps = [
                        ps_pool.tile([128, 512], F32, tag="convps", name="convps", bufs=int(_os.environ.get("K_CPS", "2")))
                        for _ in range(2)
                    ]
                    for i in range(ICCH):
                        for t in range(KER):
                            for j in range(2):
                                lc = lp * 2 + j
                                nc.tensor.matmul(
                                    ps[j][:],
                                    w_t[i][:, t, hp * 128 : (hp + 1) * 128],
                                    x_t[i][:, lc * 512 + t : lc * 512 + t + 512],
                                    start=(i == 0 and t == 0),
                                    stop=(i == ICCH - 1 and t == KER - 1),
                                )
                    for j in range(2):
                        nc.scalar.copy(y_sb[:, j * 512 : (j + 1) * 512], ps[j][:])
                        emit_transposes(y_sb, at_dst, hp, lp, j)

            def load_xw(x_dram, w_dram, x_t, w_t):
                for i in range(ICCH):
                    t = xw_pool.tile([128, L + 2], F32R, tag=f"x{i}", name=f"xt{i}", bufs=2 if i < int(_os.environ.get("K_X2", "1")) else 1)
                    nc.vector.memset(t[:, 0:1].bitcast(F32), 0.0)
                    nc.vector.memset(t[:, L + 1 : L + 2].bitcast(F32), 0.0)
                    nc.gpsimd.dma_start(t[:, 1 : L + 1], x_dram[i * 128 : (i + 1) * 128, :])
                    x_t.append(t)
                    tw = xw_pool.tile([128, KER, OC], F32R, tag=f"w{i}", name=f"wt{i}", bufs=2 if i < W2 else 1)
                    nc.gpsimd.dma_start(tw[:], w_dram[i * 128 : (i + 1) * 128])
                    w_t.append(tw)

            def attn_head(h):
                hp, hh = h // 2, h % 2
                qh = at_q[hp][:, hh].rearrange("p a b -> p (a b)")
                kh = at_k[hp][:, hh].rearrange("p a b -> p (a b)")
                for qc in range(QCH):
                    p_sb = pb_pool.tile([128, L], F32, tag="p", name="p_sb", bufs=PSB)
                    halves = []
                    for half in range(2):
                        ps_s = ps_pool.tile([128, L // 2], F32, tag="sc", name="sc", bufs=2)
                        for kc in range(2):
                            nc.tensor.matmul(
                                ps_s[:, kc * 512 : (kc + 1) * 512],
                                qh[:, qc * 128 : (qc + 1) * 128],
                                kh[:, (half * 2 + kc) * 512 : (half * 2 + kc + 1) * 512],
                                start=True,
                                stop=True,
                            )
                        scr = small.tile([128, 1], F32, tag=f"scr_{half}", name=f"escr{half}", bufs=4)
                        nc.scalar.activation(
                            p_sb[:, half * (L // 2) : (half + 1) * (L // 2)],
                            ps_s[:], EXP,
                            bias=0.0, scale=0.125,
                            accum_out=scr[:],
                        )
                        halves.append(scr)
                    den = rs_den[h][qc]
                    rec = rs_rec[h][qc]
                    nc.vector.tensor_add(den, halves[0][:], halves[1][:])
                    nc.vector.reciprocal(rec, den)
                    nc.vector.tensor_scalar_mul(p_sb[:], p_sb[:], rec)
                    nc.sync.dma_start(attn[h, qc * 128 : (qc + 1) * 128, :], p_sb[:])

            with (
                tc.tile_pool(name="xw", bufs=1) as xw_pool,
                tc.tile_pool(name="yb", bufs=1) as y_pool,
            ):
                xq_t, wq_t = [], []
                load_xw(xq, wq, xq_t, wq_t)
                conv_pass(xq_t, wq_t, at_q, 0)
                conv_pass(xq_t, wq_t, at_q, 1)
                xk_t, wk_t = [], []
                load_xw(xk, wk, xk_t, wk_t)
                conv_pass(xk_t, wk_t, at_k, 0)
                # attention for pair 0 starts while conv-K pair 1 finishes
                attn_head(0)
                conv_pass(xk_t, wk_t, at_k, 1)
                attn_head(1)
            with tc.tile_pool(name="pc", bufs=1) as pc_pool:
                v_sb = [pc_pool.tile([128, 16, DK], F32, tag=f"v{h}", name=f"vsb{h}") for h in range(NHC)]
                for h in range(NHC):
                    nc.gpsimd.dma_start(v_sb[h][:], v[h].rearrange("(n p) d -> p n d", p=128))

                def ctx_head(h):
                    vs_ps = ps_pool.tile([1, DK], F32, tag="convps", name="vs_ps", bufs=int(_os.environ.get("K_CPS", "2")))
                    for n in range(16):
                        nc.tensor.matmul(
                            vs_ps[:], ones_c[:], v_sb[h][:, n, :],
                            start=(n == 0), stop=(n == 15),
                        )
                    vs_sb = pc_pool.tile([1, DK], F32, tag="vs_sb", name="vs_sb", bufs=2)
                    nc.scalar.copy(vs_sb[:], vs_ps[:])
                    bc_ps = ps_pool.tile([128, DK], F32, tag="trps", name="bc_ps", bufs=int(_os.environ.get("K_TPS", "2")))
                    nc.tensor.matmul(bc_ps[:], ones_r[:], vs_sb[:], start=True, stop=True)
                    bc_sb = pc_pool.tile([128, DK], F32, tag="bc_sb", name="bc_sb", bufs=2)
                    nc.vector.tensor_copy(bc_sb[:], bc_ps[:])
                    ctx_sb = pc_pool.tile([128, QCH, DK], F32, tag="ctx_sb", name="ctx_sb", bufs=2)
                    for qc in range(QCH):
                        nc.vector.tensor_scalar(
                            ctx_sb[:, qc, :], bc_sb[:],
                            rs_den[h][qc], rs_rec[h][qc],
                            op0=MULT, op1=MULT,
                        )
                    nc.sync.dma_start(ctx[h].rearrange("(n p) d -> p n d", p=128), ctx_sb[:])

                attn_head(2)
                ctx_head(0)
                ctx_head(1)
                attn_head(3)
                ctx_head(2)
                ctx_head(3)

    nc.compile()
    return nc


def kernel(Q, K, V, Wq, Wk):
    Q = np.ascontiguousarray(np.asarray(Q), dtype=np.float32)
    K = np.ascontiguousarray(np.asarray(K), dtype=np.float32)
    V = np.ascontiguousarray(np.asarray(V), dtype=np.float32)
    Wq = np.asarray(Wq)
    Wk = np.asarray(Wk)

    if "nc" not in _NC_CACHE:
        _NC_CACHE["nc"] = _build_nc()
    nc = _NC_CACHE["nc"]

    Qr = Q.reshape(B, C, L)
    Kr = K.reshape(B, C, L)
    wq_t = [
        np.ascontiguousarray(np.transpose(Wq[hg * OC : (hg + 1) * OC], (1, 2, 0)), dtype=np.float32)
        for hg in range(4)
    ]
    wk_t = [
        np.ascontiguousarray(np.transpose(Wk[hg * OC : (hg + 1) * OC], (1, 2, 0)), dtype=np.float32)
        for hg in range(4)
    ]

    in_maps = []
    for c in range(8):
        b, hg = c // 4, c % 4
        in_maps.append(
            {
                "xq": Qr[b],
                "xk": Kr[b],
                "wq": wq_t[hg],
                "wk": wk_t[hg],
                "v": np.ascontiguousarray(V[b, hg * NHC : (hg + 1) * NHC]),
            }
        )

    results = run_bass_kernel_spmd(nc, in_maps, core_ids=list(range(8))).results

    attn_full = np.empty((B, H, L, L), dtype=np.float32)
    ctx_full = np.empty((B, H, L, DK), dtype=np.float32)
    for c, r in enumerate(results):
        b, hg = c // 4, c % 4
        attn_full[b, hg * NHC : (hg + 1) * NHC] = r["attn"]
        ctx_full[b, hg * NHC : (hg + 1) * NHC] = r["ctx"]
    return (ctx_full, attn_full)

